# revision 33
# baseline (speedup 1.0000x reference)
"""Trainium2 Bass kernel for a 16-head causal MHA layer.

Problem: x:[2,2048,1024] f32, wq/wk/wv/wo:[1024,1024] f32 (Linear-style
[out,in] weights), causal softmax attention with 16 heads of dim 64.

Sharding across the 8 NeuronCores: 2-way data parallel over batch x
4-way tensor parallel over heads.  Core c handles batch c//4 and the 4
heads 4*(c%4) .. 4*(c%4)+3 (feature slice of 256 rows of wq/wk/wv and
256 columns of wo).  Each core produces a partial [2048,1024] output
(its 4 heads' contribution, already projected through its wo slice);
the host sums the 4 partials per batch.

Device dataflow (all matmul inputs fp16, fp32 PSUM accumulation):
  - host uploads x transposed per batch (xT [1024, 2048] fp16) and the
    weights pre-arranged into their SBUF layouts (contiguous DMAs)
  - qT/kT = W @ xT in [feat, token] layout; v in [token, feat] layout,
    with a constant-1 column appended per head (v|1)
  - scoresT[k,q] = kT_h.T-block @ qT_h (64-dim contraction), exp on ACT
    straight out of PSUM (no max subtraction: |scores/8| < ~4 so exp is
    safe in fp32/fp16), causal mask applied only on diagonal blocks via
    a precomputed 0/1 mask multiply
  - out_unnorm.T | l = (v|1).T-block @ expT accumulated over k blocks
    (the appended ones-column yields the softmax denominator l for free)
  - softmax denominators: copy the psum l-row to sbuf (DVE; scalar ACT
    on the last chunk), 1/l via the fast-approx DVE reciprocal, cast to
    f16 on the idle gpsimd engine, broadcast over the 64 dh partitions
    with a K=1 outer-product matmul (ones[1,64].T x recip), then
    copy+scale the psum attention output into outT (copies on scalar
    for the last chunk to shorten the serial DVE tail)
  - y = outT.T @ woT accumulated over the 256-dim feature slice; output
    DMAs alternate between the sync and gpsimd queues
"""

import numpy as np

S = 2048          # sequence length (one batch per core)
D = 1024          # model dim
HL = 4            # heads handled per core
DH = 64           # head dim
F = HL * DH       # 256 local features
DC = D // 128     # 8 d_model chunks of 128
FC = F // 128     # 2 feature chunks of 128
NT = S // 128     # 16 token tiles
NQ = S // 512     # 4 query chunks of 512

_CACHE = {}


def _build_program():
    key = "nc"
    if key in _CACHE:
        return _CACHE[key]

    import concourse.bacc as bacc
    import concourse.bass as bass
    import concourse.mybir as mybir
    import concourse.tile as tile

    f16 = mybir.dt.float16
    f32 = mybir.dt.float32
    Exp = mybir.ActivationFunctionType.Exp
    Copy = mybir.ActivationFunctionType.Copy

    nc = bacc.Bacc("TRN2", target_bir_lowering=False, debug=False)

    xT_d = nc.dram_tensor("xT", [NQ, DC, 128, 512], f16, kind="ExternalInput")
    wqT_d = nc.dram_tensor("wqT", [128, DC, F], f16, kind="ExternalInput")
    wkT_d = nc.dram_tensor("wkT", [128, DC, F], f16, kind="ExternalInput")
    wvT_d = nc.dram_tensor("wvT", [128, DC, F], f16, kind="ExternalInput")
    woT_d = nc.dram_tensor("woT", [128, FC, D], f16, kind="ExternalInput")
    mask_d = nc.dram_tensor("mask", [128, 128], f16, kind="ExternalInput")
    y_d = nc.dram_tensor("y", [S, D], f16, kind="ExternalOutput")

    with tile.TileContext(nc) as tc:
        with tc.tile_pool(name="const", bufs=1) as cpool:
            xT = cpool.tile([128, DC, S], f16)
            wq = cpool.tile([128, DC, F], f16)
            wk = cpool.tile([128, DC, F], f16)
            wv = cpool.tile([128, DC, F], f16)
            wo = cpool.tile([128, FC, D], f16)
            mask = cpool.tile([128, 128], f16)
            qT = cpool.tile([128, FC, S], f16)
            kT = cpool.tile([128, FC, S], f16)
            v = cpool.tile([128, NT, HL, DH + 1], f16)
            outT = cpool.tile([128, FC, S], f16)
            ones1 = cpool.tile([1, DH], f16)
            lstage32 = cpool.tile([1, HL * S], f32)
            lrec32 = cpool.tile([1, HL * S], f32)
            lrec = cpool.tile([1, HL * S], f16)

            nc.vector.memset(ones1[:], 1.0)
            # only the softmax-denominator ones-columns; proj_v fills the rest
            nc.gpsimd.memset(v[:, :, :, DH:DH + 1], 1.0)

            # input loads over the three DMA-capable queues; weights are
            # host-pre-arranged into SBUF layout so every DMA is contiguous.
            # x is loaded in 512-token column blocks: chunk-0 projections
            # need only block 0 of every d_model chunk, so attention starts
            # while blocks 1-3 are still streaming in.
            nc.sync.dma_start(wq[:, :, 0:128], wqT_d[:, :, 0:128])
            nc.scalar.dma_start(wk[:, :, 0:128], wkT_d[:, :, 0:128])
            xq = {0: nc.sync, 3: nc.sync, 6: nc.sync,
                  1: nc.scalar, 4: nc.scalar, 7: nc.scalar,
                  2: nc.gpsimd, 5: nc.gpsimd}

            def load_xblock(b):
                for dc in range(DC):
                    xq[dc].dma_start(
                        xT[:, dc, b * 512:(b + 1) * 512], xT_d[b, dc])

            load_xblock(0)
            nc.sync.dma_start(wq[:, :, 128:256], wqT_d[:, :, 128:256])
            nc.scalar.dma_start(wk[:, :, 128:256], wkT_d[:, :, 128:256])
            nc.gpsimd.dma_start(wv[:], wvT_d[:])
            nc.sync.dma_start(mask[:], mask_d[:])
            for b in range(1, NQ):
                load_xblock(b)
            # wo is first needed by wo_tile, well after the loads above
            nc.gpsimd.dma_start(wo[:], woT_d[:])

            # ---- attention + normalize + output projection -------------
            # qc-major: all heads for query-chunk qc, then inline softmax
            # normalization; the wo projection for chunk qc runs (lagged)
            # as filler inside chunk qc+1.  The wo/proj matmuls fill the PE
            # bubbles of the exp-bound attention loop.
            with tc.tile_pool(name="ps", bufs=2,
                              space=bass.MemorySpace.PSUM) as psp, \
                 tc.tile_pool(name="sb", bufs=6) as sbp:

                # accumulation order matched to the x-chunk DMA arrival
                # order so chunk-0 projections can start as data lands
                DC_ORDER = (2, 1, 0, 5, 3, 4, 6, 7)

                def proj_qk_group(w_sb, dstT, fc, t5):
                    ps = psp.tile([128, 512], f32, tag="ybc",
                                  name=f"ps_{t5}_{fc}")
                    for i, dc in enumerate(DC_ORDER):
                        nc.tensor.matmul(
                            ps[:],
                            w_sb[:, dc, fc * 128:(fc + 1) * 128],
                            xT[:, dc, t5 * 512:(t5 + 1) * 512],
                            start=(i == 0), stop=(i == DC - 1))
                    nc.vector.tensor_copy(
                        dstT[:, fc, t5 * 512:(t5 + 1) * 512], ps[:])

                def proj_v_group(tt):
                    psv = psp.tile([128, F], f32, tag="ybc",
                                   name=f"psv_{tt}")
                    for i, dc in enumerate(DC_ORDER):
                        nc.tensor.matmul(
                            psv[:],
                            xT[:, dc, tt * 128:(tt + 1) * 128],
                            wv[:, dc, :],
                            start=(i == 0), stop=(i == DC - 1))
                    nc.vector.tensor_copy(
                        v[:, tt, :, 0:DH],
                        psv.rearrange("p (h d) -> p h d", h=HL))

                import collections
                fillers = collections.deque()

                # HAM ramp burst: the utilization limit only reaches 8/8
                # after ~4us of sustained full-array activity, and idle
                # time accumulates activity credit.  Gate the burst on the
                # first x chunk (lands ~5us into the load window) so the
                # prologue idle builds credit first and the limit is at
                # 8/8 right when the dense chunk-0 projections begin.
                wps = psp.tile([128, 512], f32, tag="ybc", name="warm_ps")
                for _ in range(10):
                    nc.tensor.matmul(
                        wps[:], wq[:, 0, 0:128],
                        wq[:, 0, 0:1].to_broadcast((128, 512)),
                        start=True, stop=True)

                def run_filler(n):
                    for _ in range(n):
                        if fillers:
                            fillers.popleft()()

                avs_store = {}

                def att_hc(qc, hc):
                    avs = []
                    for hp2 in range(2):
                        av = psp.tile([DH + 1, 512], f32, tag="av",
                                      name=f"av_{hc}_{qc}_{hp2}")
                        avs.append(av)
                    avs_store[(qc, hc)] = avs
                    for g in range(qc + 1):
                        diag = (g == qc)
                        for half in range(2):
                            # (offset, width) of each k-block's valid
                            # q-span inside the p tile; diagonal blocks
                            # are clipped to q >= k_block_start
                            if diag:
                                rs = [2 * half, 2 * half + 1]
                                spans = [(128 * r, 512 - 128 * r)
                                         for r in rs]
                            else:
                                spans = [(0, 512), (0, 512)]
                            offs = [0, spans[0][1]]
                            scs = []
                            for hp2 in range(2):
                                sc = psp.tile([128, 1024], f32, tag="sc",
                                              name=f"sc_{hc}_{qc}_{g}_{half}_{hp2}")
                                scs.append(sc)
                            for r2 in range(2):
                                kb = 4 * g + 2 * half + r2
                                qo, w = spans[r2]
                                for hp2 in range(2):
                                    hp = hp2 * 64
                                    nc.tensor.matmul(
                                        scs[hp2][:, offs[r2]:offs[r2] + w],
                                        kT[hp:hp + 64, hc,
                                           kb * 128:(kb + 1) * 128],
                                        qT[hp:hp + 64, hc,
                                           qc * 512 + qo:(qc + 1) * 512],
                                        start=True, stop=True,
                                        tile_position=(hp, 0))
                            width = offs[1] + spans[1][1]
                            for hp2 in range(2):
                                h = hc * 2 + hp2
                                p_sb = sbp.tile([128, 1024], f16,
                                                tag=f"p{hp2}",
                                                name=f"p_{hc}_{qc}_{g}_{half}_{hp2}")
                                nc.scalar.activation(
                                    p_sb[:, 0:width],
                                    scs[hp2][:, 0:width], Exp)
                                if diag:
                                    # only the first 128 columns of a
                                    # clipped block straddle the diagonal
                                    for r2 in range(2):
                                        nc.vector.tensor_mul(
                                            p_sb[:, offs[r2]:offs[r2] + 128],
                                            p_sb[:, offs[r2]:offs[r2] + 128],
                                            mask[:])
                                for r2 in range(2):
                                    kb = 4 * g + 2 * half + r2
                                    qo, w = spans[r2]
                                    nc.tensor.matmul(
                                        avs[hp2][:, qo:512],
                                        v[:, kb, h, :],
                                        p_sb[:, offs[r2]:offs[r2] + w],
                                        start=(kb == 0),
                                        stop=(kb == 4 * qc + 3))
                            run_filler(1 if qc == NQ - 1 else 2)
                    # stage this pair's softmax denominators and compute
                    # 1/l right away: for hc=0 this overlaps the hc=1
                    # attention.  In the last chunk hc=1's copies/casts go
                    # to the scalar engine (its exp work is done) so the
                    # DVE and scalar tails run in parallel.
                    on_scalar = (qc == NQ - 1 and hc == 1)
                    for hp2 in range(2):
                        h = hc * 2 + hp2
                        lseg = slice(h * S + qc * 512,
                                     h * S + (qc + 1) * 512)
                        if on_scalar:
                            nc.scalar.activation(
                                lstage32[0:1, lseg],
                                avs[hp2][DH:DH + 1, :], Copy)
                        else:
                            nc.vector.tensor_copy(
                                lstage32[0:1, lseg], avs[hp2][DH:DH + 1, :])
                        nc.vector.reciprocal_approx_fast(
                            lrec32[0:1, lseg], lstage32[0:1, lseg])
                        if on_scalar:
                            nc.scalar.activation(
                                lrec[0:1, lseg], lrec32[0:1, lseg], Copy)
                        else:
                            nc.vector.tensor_copy(
                                lrec[0:1, lseg], lrec32[0:1, lseg])

                def norm_pair(qc, hc):
                    # broadcast 1/l over the 64 dh partitions with a K=1
                    # outer-product matmul, then copy+scale the psum
                    # attention output into outT
                    avs = avs_store.pop((qc, hc))
                    seg = slice(qc * 512, (qc + 1) * 512)
                    on_scalar = (qc == NQ - 1 and hc == 1)
                    for hp2 in range(2):
                        h = hc * 2 + hp2
                        hp = hp2 * 64
                        lseg = slice(h * S + qc * 512,
                                     h * S + (qc + 1) * 512)
                        bc = psp.tile([DH, 512], f32, tag="ybc",
                                      name=f"bc_{h}_{qc}")
                        nc.tensor.matmul(
                            bc[:], ones1[:], lrec[0:1, lseg],
                            start=True, stop=True)
                        if on_scalar:
                            nc.scalar.activation(
                                outT[hp:hp + 64, hc, seg],
                                avs[hp2][0:DH, :], Copy)
                        else:
                            nc.vector.tensor_copy(
                                outT[hp:hp + 64, hc, seg], avs[hp2][0:DH, :])
                        nc.vector.tensor_mul(
                            outT[hp:hp + 64, hc, seg],
                            outT[hp:hp + 64, hc, seg], bc[:])

                def wo_tile(qt, oc, tag="ybc"):
                    yps = psp.tile([128, 512], f32, tag=tag,
                                   name=f"yps_{qt}_{oc}")
                    for fc in range(FC):
                        nc.tensor.matmul(
                            yps[:],
                            outT[:, fc, qt * 128:(qt + 1) * 128],
                            wo[:, fc, oc * 512:(oc + 1) * 512],
                            start=(fc == 0), stop=(fc == FC - 1))
                    ysb = sbp.tile([128, 512], f16, tag="ysb",
                                   name=f"ysb_{qt}_{oc}")
                    if qt >= 4 * (NQ - 1) and (qt + oc) % 2 == 0:
                        nc.scalar.activation(ysb[:], yps[:], Copy)
                    else:
                        nc.vector.tensor_copy(ysb[:], yps[:])
                    if qt >= 4 * (NQ - 1):
                        # both HWDGE rings are idle at the end and drain
                        # fast (no gpsimd q7 drain); split the final 1MB
                        # flush across them
                        q_eng = nc.sync if (2 * qt + oc) % 2 == 0 else nc.scalar
                    else:
                        q_eng = nc.sync if (2 * qt + oc) % 2 == 0 else nc.gpsimd
                    q_eng.dma_start(
                        y_d[qt * 128:(qt + 1) * 128,
                            oc * 512:(oc + 1) * 512],
                        ysb[:])

                # chunk-0 projections, fc0 first so att(0,0) can start
                # while fc1 is still projecting (as the first fillers)
                for w_sb, dstT in ((wq, qT), (wk, kT)):
                    proj_qk_group(w_sb, dstT, 0, 0)
                for tt in range(0, 4):
                    proj_v_group(tt)
                for w_sb, dstT in ((wq, qT), (wk, kT)):
                    fillers.append(
                        lambda w=w_sb, d=dstT: proj_qk_group(w, d, 1, 0))

                for qc in range(NQ):
                    if qc + 1 < NQ:
                        for w_sb, dstT in ((wq, qT), (wk, kT)):
                            for fc in range(FC):
                                fillers.append(
                                    lambda w=w_sb, d=dstT, f=fc, t=qc + 1:
                                    proj_qk_group(w, d, f, t))
                        for tt in range(4 * (qc + 1), 4 * (qc + 2)):
                            fillers.append(lambda t=tt: proj_v_group(t))
                    att_hc(qc, 0)
                    att_hc(qc, 1)
                    if qc < NQ - 1:
                        norm_pair(qc, 0)
                        norm_pair(qc, 1)
                    run_filler(len(fillers))
                    # this chunk's output projection runs as filler inside
                    # the next chunk's attention; the final chunk's tiles
                    # run directly at the end, alternating between the ybc
                    # and the (post-attention idle) sc psum rings
                    if qc < NQ - 1:
                        for qt in range(4 * qc, 4 * (qc + 1)):
                            for oc in range(2):
                                fillers.append(
                                    lambda a=qt, b=oc: wo_tile(a, b))
                def norm_qt(qc, hc, qt):
                    avs = avs_store[(qc, hc)]
                    lo = qt * 128 - qc * 512
                    for hp2 in range(2):
                        h = hc * 2 + hp2
                        hp = hp2 * 64
                        bc = psp.tile([DH, 128], f32, tag="ybc",
                                      name=f"bcq_{h}_{qt}")
                        nc.tensor.matmul(
                            bc[:], ones1[:],
                            lrec[0:1, h * S + qt * 128:
                                 h * S + (qt + 1) * 128],
                            start=True, stop=True)
                        dst = outT[hp:hp + 64, hc,
                                   qt * 128:(qt + 1) * 128]
                        if hc == 1:
                            nc.scalar.activation(
                                dst, avs[hp2][0:DH, lo:lo + 128], Copy)
                        else:
                            nc.vector.tensor_copy(
                                dst, avs[hp2][0:DH, lo:lo + 128])
                        nc.vector.tensor_mul(dst, dst, bc[:])

                for qt in range(4 * (NQ - 1), 4 * NQ):
                    norm_qt(NQ - 1, 0, qt)
                    norm_qt(NQ - 1, 1, qt)
                    for oc in range(2):
                        wo_tile(qt, oc, "sc")
                avs_store.pop((NQ - 1, 0))
                avs_store.pop((NQ - 1, 1))

    nc.compile()

    from concourse.bass_interp import get_hw_module
    nc.m = get_hw_module(nc.m)

    _CACHE[key] = nc
    return nc


def _make_mask():
    # lower-triangle 0/1 mask for the 128-wide column band of a diagonal
    # block: element (p, j) of the band is valid when q >= k, i.e. j >= p
    j = np.arange(128)[None, :]
    p = np.arange(128)[:, None]
    return (j >= p).astype(np.float16)


def kernel(x, wq, wk, wv, wo):
    x = np.asarray(x, dtype=np.float32)
    wq = np.asarray(wq, dtype=np.float32)
    wk = np.asarray(wk, dtype=np.float32)
    wv = np.asarray(wv, dtype=np.float32)
    wo = np.asarray(wo, dtype=np.float32)

    from concourse import bass_utils

    nc = _build_program()
    mask = _make_mask()

    in_maps = []
    for c in range(8):
        b = c // 4
        hg = c % 4
        fs = slice(hg * F, (hg + 1) * F)
        xT = np.ascontiguousarray(
            x[b].T.astype(np.float16).reshape(DC, 128, NQ, 512)
            .transpose(2, 0, 1, 3))
        wqT = (wq[fs, :] * 0.125).T.astype(np.float16)
        wkT = wk[fs, :].T.astype(np.float16)
        wvT = wv[fs, :].T.astype(np.float16)
        woT = wo[:, fs].T.astype(np.float16)
        in_maps.append({
            "xT": xT,
            "wqT": np.ascontiguousarray(
                wqT.reshape(DC, 128, F).transpose(1, 0, 2)),
            "wkT": np.ascontiguousarray(
                wkT.reshape(DC, 128, F).transpose(1, 0, 2)),
            "wvT": np.ascontiguousarray(
                wvT.reshape(DC, 128, F).transpose(1, 0, 2)),
            "woT": np.ascontiguousarray(
                woT.reshape(FC, 128, D).transpose(1, 0, 2)),
            "mask": mask,
        })

    res = bass_utils.run_bass_kernel_spmd(nc, in_maps, core_ids=list(range(8)))
    ys = [res.results[c]["y"].astype(np.float32) for c in range(8)]
    out = np.stack([ys[0] + ys[1] + ys[2] + ys[3],
                    ys[4] + ys[5] + ys[6] + ys[7]])
    return out


# revision 34
# speedup vs baseline: 1.0519x; 1.0519x over previous
"""Trainium2 Bass kernel for a 16-head causal MHA layer.

Problem: x:[2,2048,1024] f32, wq/wk/wv/wo:[1024,1024] f32 (Linear-style
[out,in] weights), causal softmax attention with 16 heads of dim 64.

Sharding across the 8 NeuronCores: 2-way data parallel over batch x
4-way tensor parallel over heads.  Core c handles batch c//4 and the 4
heads 4*(c%4) .. 4*(c%4)+3 (feature slice of 256 rows of wq/wk/wv and
256 columns of wo).  Each core produces a partial [2048,1024] output
(its 4 heads' contribution, already projected through its wo slice);
the host sums the 4 partials per batch.

Device dataflow (all matmul inputs fp16, fp32 PSUM accumulation):
  - host uploads x transposed per batch (xT [1024, 2048] fp16) and the
    weights pre-arranged into their SBUF layouts (contiguous DMAs)
  - qT/kT = W @ xT in [feat, token] layout; v in [token, feat] layout,
    with a constant-1 column appended per head (v|1)
  - scoresT[k,q] = kT_h.T-block @ qT_h (64-dim contraction), exp on ACT
    straight out of PSUM (no max subtraction: |scores/8| < ~4 so exp is
    safe in fp32/fp16), causal mask applied only on diagonal blocks via
    a precomputed 0/1 mask multiply
  - out_unnorm.T | l = (v|1).T-block @ expT accumulated over k blocks
    (the appended ones-column yields the softmax denominator l for free)
  - softmax denominators: copy the psum l-row to sbuf (DVE; scalar ACT
    on the last chunk), 1/l via the fast-approx DVE reciprocal, cast to
    f16 on the idle gpsimd engine, broadcast over the 64 dh partitions
    with a K=1 outer-product matmul (ones[1,64].T x recip), then
    copy+scale the psum attention output into outT (copies on scalar
    for the last chunk to shorten the serial DVE tail)
  - y = outT.T @ woT accumulated over the 256-dim feature slice; output
    DMAs alternate between the sync and gpsimd queues
"""

import numpy as np

S = 2048          # sequence length (one batch per core)
D = 1024          # model dim
HL = 4            # heads handled per core
DH = 64           # head dim
F = HL * DH       # 256 local features
DC = D // 128     # 8 d_model chunks of 128
FC = F // 128     # 2 feature chunks of 128
NT = S // 128     # 16 token tiles
NQ = S // 512     # 4 query chunks of 512

_CACHE = {}


def _build_program():
    key = "nc"
    if key in _CACHE:
        return _CACHE[key]

    import concourse.bacc as bacc
    import concourse.bass as bass
    import concourse.mybir as mybir
    import concourse.tile as tile

    f16 = mybir.dt.float16
    f32 = mybir.dt.float32
    Exp = mybir.ActivationFunctionType.Exp
    Copy = mybir.ActivationFunctionType.Copy

    nc = bacc.Bacc("TRN2", target_bir_lowering=False, debug=False)

    xT_d = nc.dram_tensor("xT", [NQ, DC, 128, 512], f16, kind="ExternalInput")
    wqT_d = nc.dram_tensor("wqT", [128, DC, F], f16, kind="ExternalInput")
    wkT_d = nc.dram_tensor("wkT", [128, DC, F], f16, kind="ExternalInput")
    wvT_d = nc.dram_tensor("wvT", [128, DC, F], f16, kind="ExternalInput")
    woT_d = nc.dram_tensor("woT", [128, FC, D], f16, kind="ExternalInput")
    mask_d = nc.dram_tensor("mask", [128, 128], f16, kind="ExternalInput")
    y_d = nc.dram_tensor("y", [S, D], f16, kind="ExternalOutput")

    with tile.TileContext(nc) as tc:
        with tc.tile_pool(name="const", bufs=1) as cpool:
            xT = cpool.tile([128, DC, S], f16)
            wq = cpool.tile([128, DC, F], f16)
            wk = cpool.tile([128, DC, F], f16)
            wv = cpool.tile([128, DC, F], f16)
            wo = cpool.tile([128, FC, D], f16)
            mask = cpool.tile([128, 128], f16)
            qT = cpool.tile([128, FC, S], f16)
            kT = cpool.tile([128, FC, S], f16)
            v = cpool.tile([128, NT, HL, DH + 1], f16)
            outT = cpool.tile([128, FC, S], f16)
            ones1 = cpool.tile([1, DH], f16)
            lstage32 = cpool.tile([1, HL * S], f32)
            lrec32 = cpool.tile([1, HL * S], f32)
            lrec = cpool.tile([1, HL * S], f16)

            nc.vector.memset(ones1[:], 1.0)
            # only the softmax-denominator ones-columns; proj_v fills the rest
            nc.gpsimd.memset(v[:, :, :, DH:DH + 1], 1.0)

            # input loads over the three DMA-capable queues; weights are
            # host-pre-arranged into SBUF layout so every DMA is contiguous.
            # x is loaded in 512-token column blocks: chunk-0 projections
            # need only block 0 of every d_model chunk, so attention starts
            # while blocks 1-3 are still streaming in.
            nc.sync.dma_start(wq[:, :, 0:128], wqT_d[:, :, 0:128])
            nc.scalar.dma_start(wk[:, :, 0:128], wkT_d[:, :, 0:128])
            xq = {0: nc.sync, 3: nc.sync, 6: nc.sync,
                  1: nc.scalar, 4: nc.scalar, 7: nc.scalar,
                  2: nc.gpsimd, 5: nc.gpsimd}

            def load_xblock(b):
                for dc in range(DC):
                    xq[dc].dma_start(
                        xT[:, dc, b * 512:(b + 1) * 512], xT_d[b, dc])

            load_xblock(0)
            nc.sync.dma_start(wq[:, :, 128:256], wqT_d[:, :, 128:256])
            nc.scalar.dma_start(wk[:, :, 128:256], wkT_d[:, :, 128:256])
            nc.gpsimd.dma_start(wv[:], wvT_d[:])
            nc.sync.dma_start(mask[:], mask_d[:])
            for b in range(1, NQ):
                load_xblock(b)
            # wo is first needed by wo_tile, well after the loads above
            nc.gpsimd.dma_start(wo[:], woT_d[:])

            # ---- attention + normalize + output projection -------------
            # qc-major: all heads for query-chunk qc, then inline softmax
            # normalization; the wo projection for chunk qc runs (lagged)
            # as filler inside chunk qc+1.  The wo/proj matmuls fill the PE
            # bubbles of the exp-bound attention loop.
            with tc.tile_pool(name="ps", bufs=2,
                              space=bass.MemorySpace.PSUM) as psp, \
                 tc.tile_pool(name="sb", bufs=6) as sbp:

                # accumulation order matched to the x-chunk DMA arrival
                # order so chunk-0 projections can start as data lands
                DC_ORDER = (2, 1, 0, 5, 3, 4, 6, 7)

                def proj_qk_group(w_sb, dstT, fc, t5):
                    ps = psp.tile([128, 512], f32, tag="ybc",
                                  name=f"ps_{t5}_{fc}")
                    for i, dc in enumerate(DC_ORDER):
                        nc.tensor.matmul(
                            ps[:],
                            w_sb[:, dc, fc * 128:(fc + 1) * 128],
                            xT[:, dc, t5 * 512:(t5 + 1) * 512],
                            start=(i == 0), stop=(i == DC - 1))
                    nc.vector.tensor_copy(
                        dstT[:, fc, t5 * 512:(t5 + 1) * 512], ps[:])

                def proj_v_group(tt):
                    psv = psp.tile([128, F], f32, tag="ybc",
                                   name=f"psv_{tt}")
                    for i, dc in enumerate(DC_ORDER):
                        nc.tensor.matmul(
                            psv[:],
                            xT[:, dc, tt * 128:(tt + 1) * 128],
                            wv[:, dc, :],
                            start=(i == 0), stop=(i == DC - 1))
                    nc.vector.tensor_copy(
                        v[:, tt, :, 0:DH],
                        psv.rearrange("p (h d) -> p h d", h=HL))

                import collections
                fillers = collections.deque()

                # HAM ramp burst: the utilization limit only reaches 8/8
                # after ~4us of sustained full-array activity, and idle
                # time accumulates activity credit.  Gate the burst on the
                # first x chunk (lands ~5us into the load window) so the
                # prologue idle builds credit first and the limit is at
                # 8/8 right when the dense chunk-0 projections begin.
                wps = psp.tile([128, 512], f32, tag="ybc", name="warm_ps")
                for _ in range(10):
                    nc.tensor.matmul(
                        wps[:], wq[:, 0, 0:128],
                        wq[:, 0, 0:1].to_broadcast((128, 512)),
                        start=True, stop=True)

                def run_filler(n):
                    for _ in range(n):
                        if fillers:
                            fillers.popleft()()

                avs_store = {}

                def att_hc(qc, hc):
                    avs = []
                    for hp2 in range(2):
                        av = psp.tile([DH + 1, 512], f32, tag="av",
                                      name=f"av_{hc}_{qc}_{hp2}")
                        avs.append(av)
                    avs_store[(qc, hc)] = avs
                    for g in range(qc + 1):
                        diag = (g == qc)
                        for half in range(2):
                            # (offset, width) of each k-block's valid
                            # q-span inside the p tile; diagonal blocks
                            # are clipped to q >= k_block_start
                            if diag:
                                rs = [2 * half, 2 * half + 1]
                                spans = [(128 * r, 512 - 128 * r)
                                         for r in rs]
                            else:
                                spans = [(0, 512), (0, 512)]
                            offs = [0, spans[0][1]]
                            scs = []
                            for hp2 in range(2):
                                sc = psp.tile([128, 1024], f32, tag="sc",
                                              name=f"sc_{hc}_{qc}_{g}_{half}_{hp2}")
                                scs.append(sc)
                            for r2 in range(2):
                                kb = 4 * g + 2 * half + r2
                                qo, w = spans[r2]
                                for hp2 in range(2):
                                    hp = hp2 * 64
                                    nc.tensor.matmul(
                                        scs[hp2][:, offs[r2]:offs[r2] + w],
                                        kT[hp:hp + 64, hc,
                                           kb * 128:(kb + 1) * 128],
                                        qT[hp:hp + 64, hc,
                                           qc * 512 + qo:(qc + 1) * 512],
                                        start=True, stop=True,
                                        tile_position=(hp, 0))
                            width = offs[1] + spans[1][1]
                            for hp2 in range(2):
                                h = hc * 2 + hp2
                                p_sb = sbp.tile([128, 1024], f16,
                                                tag=f"p{hp2}",
                                                name=f"p_{hc}_{qc}_{g}_{half}_{hp2}")
                                nc.scalar.activation(
                                    p_sb[:, 0:width],
                                    scs[hp2][:, 0:width], Exp)
                                if diag:
                                    # only the first 128 columns of a
                                    # clipped block straddle the diagonal
                                    for r2 in range(2):
                                        nc.vector.tensor_mul(
                                            p_sb[:, offs[r2]:offs[r2] + 128],
                                            p_sb[:, offs[r2]:offs[r2] + 128],
                                            mask[:])
                                for r2 in range(2):
                                    kb = 4 * g + 2 * half + r2
                                    qo, w = spans[r2]
                                    nc.tensor.matmul(
                                        avs[hp2][:, qo:512],
                                        v[:, kb, h, :],
                                        p_sb[:, offs[r2]:offs[r2] + w],
                                        start=(kb == 0),
                                        stop=(kb == 4 * qc + 3))
                            run_filler(1 if qc == NQ - 1 else 2)
                    # stage this pair's softmax denominators and compute
                    # 1/l right away: for hc=0 this overlaps the hc=1
                    # attention.  In the last chunk hc=1's copies/casts go
                    # to the scalar engine (its exp work is done) so the
                    # DVE and scalar tails run in parallel.
                    on_scalar = (qc == NQ - 1 and hc == 1)
                    for hp2 in range(2):
                        h = hc * 2 + hp2
                        lseg = slice(h * S + qc * 512,
                                     h * S + (qc + 1) * 512)
                        if on_scalar:
                            nc.scalar.activation(
                                lstage32[0:1, lseg],
                                avs[hp2][DH:DH + 1, :], Copy)
                        else:
                            nc.vector.tensor_copy(
                                lstage32[0:1, lseg], avs[hp2][DH:DH + 1, :])
                        nc.vector.reciprocal_approx_fast(
                            lrec32[0:1, lseg], lstage32[0:1, lseg])
                        if on_scalar:
                            nc.scalar.activation(
                                lrec[0:1, lseg], lrec32[0:1, lseg], Copy)
                        else:
                            nc.vector.tensor_copy(
                                lrec[0:1, lseg], lrec32[0:1, lseg])

                def norm_pair(qc, hc):
                    # broadcast 1/l over the 64 dh partitions with a K=1
                    # outer-product matmul, then copy+scale the psum
                    # attention output into outT
                    avs = avs_store.pop((qc, hc))
                    seg = slice(qc * 512, (qc + 1) * 512)
                    on_scalar = (qc == NQ - 1 and hc == 1)
                    for hp2 in range(2):
                        h = hc * 2 + hp2
                        hp = hp2 * 64
                        lseg = slice(h * S + qc * 512,
                                     h * S + (qc + 1) * 512)
                        bc = psp.tile([DH, 512], f32, tag="ybc",
                                      name=f"bc_{h}_{qc}")
                        nc.tensor.matmul(
                            bc[:], ones1[:], lrec[0:1, lseg],
                            start=True, stop=True)
                        if on_scalar:
                            nc.scalar.activation(
                                outT[hp:hp + 64, hc, seg],
                                avs[hp2][0:DH, :], Copy)
                        else:
                            nc.vector.tensor_copy(
                                outT[hp:hp + 64, hc, seg], avs[hp2][0:DH, :])
                        nc.vector.tensor_mul(
                            outT[hp:hp + 64, hc, seg],
                            outT[hp:hp + 64, hc, seg], bc[:])

                def wo_tile(qt, oc, tag="ybc"):
                    yps = psp.tile([128, 512], f32, tag=tag,
                                   name=f"yps_{qt}_{oc}")
                    for fc in range(FC):
                        nc.tensor.matmul(
                            yps[:],
                            outT[:, fc, qt * 128:(qt + 1) * 128],
                            wo[:, fc, oc * 512:(oc + 1) * 512],
                            start=(fc == 0), stop=(fc == FC - 1))
                    ysb = sbp.tile([128, 512], f16, tag="ysb",
                                   name=f"ysb_{qt}_{oc}")
                    if qt >= 4 * (NQ - 1) and (qt + oc) % 2 == 0:
                        nc.scalar.activation(ysb[:], yps[:], Copy)
                    else:
                        nc.vector.tensor_copy(ysb[:], yps[:])
                    if qt >= 4 * (NQ - 1):
                        # both HWDGE rings are idle at the end and drain
                        # fast (no gpsimd q7 drain); split the final 1MB
                        # flush across them
                        q_eng = nc.sync if (2 * qt + oc) % 2 == 0 else nc.scalar
                    else:
                        q_eng = nc.sync if (2 * qt + oc) % 2 == 0 else nc.gpsimd
                    q_eng.dma_start(
                        y_d[qt * 128:(qt + 1) * 128,
                            oc * 512:(oc + 1) * 512],
                        ysb[:])

                # chunk-0 projections, fc0 first so att(0,0) can start
                # while fc1 is still projecting (as the first fillers)
                for w_sb, dstT in ((wq, qT), (wk, kT)):
                    proj_qk_group(w_sb, dstT, 0, 0)
                for tt in range(0, 4):
                    proj_v_group(tt)
                for w_sb, dstT in ((wq, qT), (wk, kT)):
                    fillers.append(
                        lambda w=w_sb, d=dstT: proj_qk_group(w, d, 1, 0))

                for qc in range(NQ):
                    if qc + 1 < NQ:
                        for w_sb, dstT in ((wq, qT), (wk, kT)):
                            for fc in range(FC):
                                fillers.append(
                                    lambda w=w_sb, d=dstT, f=fc, t=qc + 1:
                                    proj_qk_group(w, d, f, t))
                        for tt in range(4 * (qc + 1), 4 * (qc + 2)):
                            fillers.append(lambda t=tt: proj_v_group(t))
                    att_hc(qc, 0)
                    att_hc(qc, 1)
                    norm_pair(qc, 0)
                    norm_pair(qc, 1)
                    run_filler(len(fillers))
                    # this chunk's output projection runs as filler inside
                    # the next chunk's attention; the final chunk's tiles
                    # run directly at the end, alternating between the ybc
                    # and the (post-attention idle) sc psum rings
                    if qc < NQ - 1:
                        for qt in range(4 * qc, 4 * (qc + 1)):
                            for oc in range(2):
                                fillers.append(
                                    lambda a=qt, b=oc: wo_tile(a, b))
                for i, qt in enumerate(range(4 * (NQ - 1), 4 * NQ)):
                    for oc in range(2):
                        wo_tile(qt, oc, "ybc" if (2 * i + oc) % 2 == 0
                                else "sc")

    nc.compile()

    from concourse.bass_interp import get_hw_module
    nc.m = get_hw_module(nc.m)

    _CACHE[key] = nc
    return nc


def _make_mask():
    # lower-triangle 0/1 mask for the 128-wide column band of a diagonal
    # block: element (p, j) of the band is valid when q >= k, i.e. j >= p
    j = np.arange(128)[None, :]
    p = np.arange(128)[:, None]
    return (j >= p).astype(np.float16)


def kernel(x, wq, wk, wv, wo):
    x = np.asarray(x, dtype=np.float32)
    wq = np.asarray(wq, dtype=np.float32)
    wk = np.asarray(wk, dtype=np.float32)
    wv = np.asarray(wv, dtype=np.float32)
    wo = np.asarray(wo, dtype=np.float32)

    from concourse import bass_utils

    nc = _build_program()
    mask = _make_mask()

    in_maps = []
    for c in range(8):
        b = c // 4
        hg = c % 4
        fs = slice(hg * F, (hg + 1) * F)
        xT = np.ascontiguousarray(
            x[b].T.astype(np.float16).reshape(DC, 128, NQ, 512)
            .transpose(2, 0, 1, 3))
        wqT = (wq[fs, :] * 0.125).T.astype(np.float16)
        wkT = wk[fs, :].T.astype(np.float16)
        wvT = wv[fs, :].T.astype(np.float16)
        woT = wo[:, fs].T.astype(np.float16)
        in_maps.append({
            "xT": xT,
            "wqT": np.ascontiguousarray(
                wqT.reshape(DC, 128, F).transpose(1, 0, 2)),
            "wkT": np.ascontiguousarray(
                wkT.reshape(DC, 128, F).transpose(1, 0, 2)),
            "wvT": np.ascontiguousarray(
                wvT.reshape(DC, 128, F).transpose(1, 0, 2)),
            "woT": np.ascontiguousarray(
                woT.reshape(FC, 128, D).transpose(1, 0, 2)),
            "mask": mask,
        })

    res = bass_utils.run_bass_kernel_spmd(nc, in_maps, core_ids=list(range(8)))
    ys = [res.results[c]["y"].astype(np.float32) for c in range(8)]
    out = np.stack([ys[0] + ys[1] + ys[2] + ys[3],
                    ys[4] + ys[5] + ys[6] + ys[7]])
    return out
